# revision 11
# baseline (speedup 1.0000x reference)
"""GAU Trainium2 Bass kernel, 8-core sequence-parallel, v3.

v3 over v2:
- phase-1 GEMM operands in bf16 (x_ownT, W_hidden, W_qk host-converted);
  same PE rate, half the SBUF/DMA.
- k/q in bf16 (halves the k AllGather and kt reload traffic).
- phase 2 processes BOTH i-chunks per loaded tile: kt_sb loaded once per jg,
  vt loaded once per (h-group, jp) and its DoubleRow weight load amortized
  over 2 matmuls (ic0, ic1).
- gated + W_out in bf16; W_out SBUF-resident for the whole kernel.
- attn fp8e4 x64 (x8 folded into q affine, W_out/64), v fp8e4; attn@v runs
  fp8 DoubleRow over j-tile pairs.
"""

import numpy as np
import ml_dtypes

import concourse.bass as bass
import concourse.mybir as mybir
import concourse.tile as tile
from concourse import bacc

N = 8192          # total rows
D = 1024          # model dim
QK = 200          # qk dim
H = 2048          # hidden (v/gate) dim
NC = 8            # cores
R = N // NC       # rows per core
DT = D // 128     # d-tiles
JGS = N // 512    # j-groups of 512
HT = H // 128     # h-tiles
IC = R // 512     # i-chunks per core
JP = N // 256     # j-tile pairs

f32 = mybir.dt.float32
f32r = mybir.dt.float32r
bf16 = mybir.dt.bfloat16
fp8 = mybir.dt.float8e4
ACT = mybir.ActivationFunctionType
ALU = mybir.AluOpType
DR = mybir.MatmulPerfMode.DoubleRow
GROUPS = [list(range(NC))]


def _build_nc(reps=1, vbias=False, obias=False, do_p2=True,
              gather_input=False, do_pB=True, do_pC=True):
    nc = bacc.Bacc("TRN2", target_bir_lowering=False, debug=False,
                   num_devices=NC)
    if gather_input:
        kT_gi = nc.dram_tensor("kT_gi", [NC, 2, 128, R], bf16,
                               kind="ExternalInput").ap()
        v_gi = nc.dram_tensor("v_gi", [NC, R, H], fp8,
                              kind="ExternalInput").ap()

    xoT = nc.dram_tensor("xoT", [D, R], bf16, kind="ExternalInput").ap()
    x_own = nc.dram_tensor("x_own", [R, D], f32, kind="ExternalInput").ap()
    w_h = nc.dram_tensor("w_h", [D, 2 * H], bf16, kind="ExternalInput").ap()
    w_qk = nc.dram_tensor("w_qk", [D, QK], bf16, kind="ExternalInput").ap()
    w_out = nc.dram_tensor("w_out", [H, D], bf16, kind="ExternalInput").ap()
    # per-c scalars, padded 200 -> [2, 128]
    gq = nc.dram_tensor("gq", [2, 128], f32, kind="ExternalInput").ap()
    bq = nc.dram_tensor("bq", [2, 128], f32, kind="ExternalInput").ap()
    gk = nc.dram_tensor("gk", [2, 128], f32, kind="ExternalInput").ap()
    bk = nc.dram_tensor("bk", [2, 128], f32, kind="ExternalInput").ap()
    bqk = nc.dram_tensor("bqk", [2, 128], f32, kind="ExternalInput").ap()
    bg = nc.dram_tensor("bg", [HT, 128], f32, kind="ExternalInput").ap()
    if vbias:
        bv = nc.dram_tensor("bv", [H], f32, kind="ExternalInput").ap()
    if obias:
        bo = nc.dram_tensor("bo", [D], f32, kind="ExternalInput").ap()
    out = nc.dram_tensor("out", [R, D], f32, kind="ExternalOutput").ap()

    with tile.TileContext(nc) as tc:
        with (
            tc.tile_pool(name="pers", bufs=1) as pers,
            tc.tile_pool(name="dram", bufs=1, space="DRAM") as dpool,
        ):
            # persistent small tiles
            gq_t = pers.tile([128, 2], f32)
            bq_t = pers.tile([128, 2], f32)
            gk_t = pers.tile([128, 2], f32)
            bk_t = pers.tile([128, 2], f32)
            bqk_t = pers.tile([128, 2], f32)
            bg_t = pers.tile([128, HT], f32)
            nc.sync.dma_start(out=gq_t, in_=gq.rearrange("ct c -> c ct"))
            nc.sync.dma_start(out=bq_t, in_=bq.rearrange("ct c -> c ct"))
            nc.sync.dma_start(out=gk_t, in_=gk.rearrange("ct c -> c ct"))
            nc.sync.dma_start(out=bk_t, in_=bk.rearrange("ct c -> c ct"))
            nc.sync.dma_start(out=bqk_t, in_=bqk.rearrange("ct c -> c ct"))
            nc.sync.dma_start(out=bg_t, in_=bg.rearrange("ht c -> c ht"))
            if vbias:
                bv_t = pers.tile([128, H], f32)
                nc.sync.dma_start(
                    out=bv_t,
                    in_=bass.AP(tensor=bv.tensor, offset=bv.offset,
                                ap=[[0, 128]] + list(bv.ap)),
                )
            if obias:
                bo_t = pers.tile([128, D], f32)
                nc.sync.dma_start(
                    out=bo_t,
                    in_=bass.AP(tensor=bo.tensor, offset=bo.offset,
                                ap=[[0, 128]] + list(bo.ap)),
                )

            # W_out stays in SBUF for the whole kernel (32 KB/partition)
            wo_r = w_out.rearrange("(ht p) m -> p ht m", p=128)
            wo_t = pers.tile([128, HT, D], bf16, tag="wo_t", name="wo_t")
            for dh in range(2):
                eng = nc.sync if dh % 2 else nc.gpsimd
                eng.dma_start(out=wo_t[:, dh * 8:(dh + 1) * 8, :],
                              in_=wo_r[:, dh * 8:(dh + 1) * 8, :])

            # DRAM scratch (local)
            k_b = dpool.tile([2, 128, R], bf16, tag="k_b")
            v_b = dpool.tile([R, H], fp8, tag="v_b")
            # qT and gateT live in SBUF for the whole kernel
            qT_s = pers.tile([128, 2, R], bf16, tag="qT_s", name="qT_s")
            gT_s = pers.tile([128, HT, R], bf16, tag="gT_s", name="gT_s")

            wh_r = w_h.rearrange("(dt p) h -> p dt h", p=128)
            wqk_r = w_qk.rearrange("(dt p) c -> p dt c", p=128)
            xoT_r = xoT.rearrange("(dt p) j -> p dt j", p=128)
            xo_r = x_own.rearrange("(ic it p) m -> p ic it m", p=128, it=4)

            for rep in range(reps):
                if rep:
                    # full barrier between timing reps so SBUF/PSUM region
                    # reuse across the rep boundary is strictly ordered
                    tc.strict_bb_all_engine_barrier()

                # gathered tensors: single-writer per rep
                if gather_input:
                    kT_g, v_g = kT_gi, v_gi
                else:
                    kT_g = dpool.tile([NC, 2, 128, R], bf16,
                                      addr_space="Shared",
                                      tag=f"kT_g{rep}", name=f"kT_g{rep}")
                    v_g = dpool.tile([NC, R, H], fp8, addr_space="Shared",
                                     tag=f"v_g{rep}", name=f"v_g{rep}")

                # ============ phase 1: own-row k/q, v, gate + gathers ======
                with (
                    tc.tile_pool(name="whp", bufs=1) as whp,
                    tc.tile_pool(name="st1", bufs=(2 if vbias else 3)) as st1,
                    tc.tile_pool(name="ps_qk", bufs=2, space="PSUM") as ps_qk,
                    tc.tile_pool(name="ps_v", bufs=2, space="PSUM") as ps_v,
                    tc.tile_pool(name="ps_g", bufs=2, space="PSUM") as ps_g,
                ):
                    xo_t = whp.tile([128, DT, R], bf16, tag="xo_t")
                    for dh in range(2):
                        eng = nc.sync if dh % 2 else nc.gpsimd
                        eng.dma_start(out=xo_t[:, dh * 4:(dh + 1) * 4, :],
                                      in_=xoT_r[:, dh * 4:(dh + 1) * 4, :])
                    wqk_t = whp.tile([128, DT, QK], bf16, tag="wqk")
                    nc.sync.dma_start(out=wqk_t, in_=wqk_r)
                    wh_t = whp.tile([128, DT, 2 * H], bf16, tag="wh")
                    for dt in range(DT):
                        eng = nc.sync if dt % 2 else nc.gpsimd
                        eng.dma_start(out=wh_t[:, dt, :], in_=wh_r[:, dt, :])

                    # ---- qk -> k_own (bounce), q (own) ----
                    for jg in range(IC):
                        for ct in range(2):
                            cw = 128 if ct == 0 else QK - 128
                            pq = ps_qk.tile([128, 512], f32)
                            for dt in range(DT):
                                nc.tensor.matmul(
                                    pq[:cw],
                                    wqk_t[:, dt, ct * 128:ct * 128 + cw],
                                    xo_t[:, dt, jg * 512:(jg + 1) * 512],
                                    start=(dt == 0),
                                    stop=(dt == DT - 1),
                                )
                            sil = st1.tile([128, 512], f32, tag="sil")
                            nc.scalar.activation(
                                sil[:cw], pq[:cw], ACT.Silu,
                                bias=bqk_t[:cw, ct:ct + 1],
                            )
                            kt = st1.tile([128, 512], bf16, tag="kt")
                            nc.vector.tensor_scalar(
                                out=kt[:cw], in0=sil[:cw],
                                scalar1=gk_t[:cw, ct:ct + 1],
                                scalar2=bk_t[:cw, ct:ct + 1],
                                op0=ALU.mult, op1=ALU.add,
                            )
                            nc.sync.dma_start(
                                out=k_b[ct, 0:cw, jg * 512:(jg + 1) * 512],
                                in_=kt[:cw],
                            )
                            nc.vector.tensor_scalar(
                                out=qT_s[:cw, ct, jg * 512:(jg + 1) * 512],
                                in0=sil[:cw],
                                scalar1=gq_t[:cw, ct:ct + 1],
                                scalar2=bq_t[:cw, ct:ct + 1],
                                op0=ALU.mult, op1=ALU.add,
                            )
                    if not gather_input:
                        nc.gpsimd.collective_compute(
                            "AllGather", ALU.bypass, replica_groups=GROUPS,
                            ins=[k_b.opt()], outs=[kT_g.opt()],
                        )

                    # ---- v (own rows) ----
                    for it in range(R // 128):
                        for hc in range(4):
                            pv = ps_v.tile([128, 512], f32)
                            for dt in range(DT):
                                nc.tensor.matmul(
                                    pv,
                                    xo_t[:, dt, it * 128:(it + 1) * 128],
                                    wh_t[:, dt, hc * 512:(hc + 1) * 512],
                                    start=(dt == 0),
                                    stop=(dt == DT - 1),
                                )
                            vt = st1.tile([128, 512], fp8, tag="vt")
                            if vbias:
                                tmp = st1.tile([128, 512], f32, tag="vtmp")
                                nc.vector.tensor_add(
                                    tmp, pv, bv_t[:, hc * 512:(hc + 1) * 512])
                                nc.scalar.activation(vt, tmp, ACT.Silu)
                            else:
                                nc.scalar.activation(vt, pv, ACT.Silu)
                            veng = nc.sync if (it + hc) % 2 else nc.gpsimd
                            veng.dma_start(
                                out=v_b[it * 128:(it + 1) * 128,
                                        hc * 512:(hc + 1) * 512],
                                in_=vt,
                            )
                    if not gather_input:
                        nc.gpsimd.collective_compute(
                            "AllGather", ALU.bypass, replica_groups=GROUPS,
                            ins=[v_b.opt()], outs=[v_g.opt()],
                        )

                    # ---- gateT (own rows) -> SBUF-resident bf16 ----
                    for jg in range(IC):
                        for ht in range(HT):
                            pg = ps_g.tile([128, 512], f32)
                            for dt in range(DT):
                                nc.tensor.matmul(
                                    pg,
                                    wh_t[:, dt, H + ht * 128:H + (ht + 1) * 128],
                                    xo_t[:, dt, jg * 512:(jg + 1) * 512],
                                    start=(dt == 0),
                                    stop=(dt == DT - 1),
                                )
                            nc.scalar.activation(
                                gT_s[:, ht, jg * 512:(jg + 1) * 512],
                                pg, ACT.Silu, bias=bg_t[:, ht:ht + 1])

                # ================= phase 2: attention, both i-chunks =======
                if not do_p2:
                    # DCE anchor: touch gathered outputs + gT_d -> out
                    with tc.tile_pool(name="probe", bufs=1) as prp:
                        pt = prp.tile([128, 512], f32, tag="pt", name="pt")
                        nc.sync.dma_start(
                            out=pt, in_=v_g[NC - 1, 0:128, 0:2048]
                            .bitcast(f32))
                        pt2 = prp.tile([128, 512], f32, tag="pt2", name="pt2")
                        nc.sync.dma_start(
                            out=pt2, in_=kT_g[NC - 1, 0, :, 0:1024]
                            .bitcast(f32))
                        nc.vector.tensor_add(pt, pt, pt2)
                        pt3 = prp.tile([128, 512], f32, tag="pt3", name="pt3")
                        nc.sync.dma_start(out=pt3, in_=gT_d[0, :, 0:512])
                        nc.vector.tensor_add(pt, pt, pt3)
                        nc.sync.dma_start(
                            out=out.rearrange("(a p) m -> p a m", p=128)
                            [:, rep % 8, 0:512], in_=pt)
                    continue
                with (
                    tc.tile_pool(name="p2sb", bufs=1) as p2sb,
                    tc.tile_pool(name="kqp", bufs=2) as kqp,
                    tc.tile_pool(name="vst", bufs=10) as vst,
                    tc.tile_pool(name="xop", bufs=2) as xop,
                    tc.tile_pool(name="ost", bufs=2) as osp,
                ):
                    attns = [p2sb.tile([128, N // 128, 512], fp8,
                                       tag=f"attn{i}", name=f"attn{i}")
                             for i in range(IC)]
                    gateds = [p2sb.tile([128, HT, 512], bf16,
                                        tag=f"gated{i}", name=f"gated{i}")
                              for i in range(IC)]

                    # ---- A: attn[j, i] = relu(k.T q)^2, both i-chunks ----
                    with tc.tile_pool(name="psA", bufs=1, space="PSUM") as psA:
                        for jg in range(JGS):
                            kt_sb = kqp.tile([128, 2, 512], bf16,
                                             tag="kt_sb", name="kt_sb")
                            keng = nc.sync if jg % 2 else nc.gpsimd
                            keng.dma_start(
                                out=kt_sb,
                                in_=kT_g[jg // 2, :, :,
                                         (jg % 2) * 512:(jg % 2 + 1) * 512]
                                .rearrange("ct c j -> c ct j"),
                            )
                            for ic in range(IC):
                                q_sb = qT_s[:, :, ic * 512:(ic + 1) * 512]
                                for j4 in range(4):
                                    jt = jg * 4 + j4
                                    pss = psA.tile([128, 512], f32, tag="sim",
                                                   bufs=3, name="pss")
                                    nc.tensor.matmul(
                                        pss,
                                        kt_sb[:, 0, j4 * 128:(j4 + 1) * 128],
                                        q_sb[:, 0, :], start=True, stop=False)
                                    nc.tensor.matmul(
                                        pss, kt_sb[0:QK - 128, 1,
                                                   j4 * 128:(j4 + 1) * 128],
                                        q_sb[0:QK - 128, 1, :],
                                        start=False, stop=True)
                                    rel = kqp.tile([128, 512], f32,
                                                   tag="rel", bufs=3,
                                                   name="rel")
                                    nc.scalar.activation(rel, pss, ACT.Relu)
                                    nc.vector.tensor_mul(
                                        attns[ic][:, jt, :], rel, rel)

                    if not do_pB:
                        # anchor: attn -> out row block
                        pa = kqp.tile([128, 128], f32, tag="pa", bufs=1,
                                      name="pa")
                        nc.vector.tensor_copy(
                            pa, attns[0][:, 4 * (rep % 16), :].bitcast(f32))
                        nc.sync.dma_start(
                            out=out.rearrange("(a p) m -> p a m", p=128)
                            [:, rep % 8, 0:128], in_=pa)
                        continue

                    # ---- B: out1T[h, i] = v-lhsT @ attn (fp8 DoubleRow),
                    #         vt shared across i-chunks; * gateT ----
                    # per-bank accumulator tiles: bank reuse across h-groups
                    # pipelines per-region instead of stalling on the whole
                    # group's readout
                    with tc.tile_pool(name="psB", bufs=1, space="PSUM") as psB:
                        for g in range(4):
                            po = [psB.tile([128, 512], f32, tag=f"po{j}",
                                           name=f"po{j}")
                                  for j in range(8)]
                            for jp in range(JP):
                                vt = vst.tile([128, 2, 512], fp8,
                                              tag="vt", name="vt")
                                veng = nc.sync if jp % 2 else nc.gpsimd
                                veng.dma_start(
                                    out=vt,
                                    in_=v_g[jp // 4,
                                            (jp % 4) * 256:(jp % 4 + 1) * 256,
                                            g * 512:(g + 1) * 512]
                                    .rearrange("(two p) h -> p two h", two=2),
                                )
                                for hh in range(4):
                                    for ic in range(IC):
                                        nc.tensor.matmul(
                                            po[hh * 2 + ic],
                                            vt[:, :, hh * 128:(hh + 1) * 128],
                                            attns[ic][:, 2 * jp:2 * jp + 2, :],
                                            start=(jp == 0),
                                            stop=(jp == JP - 1),
                                            perf_mode=DR,
                                        )
                            for hh in range(4):
                                ht = g * 4 + hh
                                for ic in range(IC):
                                    nc.vector.tensor_mul(
                                        gateds[ic][:, ht, :],
                                        po[hh * 2 + ic],
                                        gT_s[:, ht,
                                             ic * 512:(ic + 1) * 512])

                    if not do_pC:
                        pb = kqp.tile([128, 256], f32, tag="pb", bufs=1,
                                      name="pb")
                        nc.vector.tensor_copy(
                            pb, gateds[0][:, rep % 16, :].bitcast(f32))
                        nc.sync.dma_start(
                            out=out.rearrange("(a p) m -> p a m", p=128)
                            [:, rep % 8, 0:256], in_=pb)
                        continue

                    # ---- C: out2 = gatedT.T @ W_out; out = out2 * x ----
                    # each gated weight load feeds both m-halves (2 matmuls)
                    with tc.tile_pool(name="psC", bufs=1, space="PSUM") as psC:
                        for ic in range(IC):
                            pos = [psC.tile([128, 512], f32, tag=f"pc{j}",
                                            name=f"pc{j}")
                                   for j in range(8)]
                            for ht in range(HT):
                                for it in range(4):
                                    for mc in range(2):
                                        nc.tensor.matmul(
                                            pos[it * 2 + mc],
                                            gateds[ic][:, ht,
                                                       it * 128:
                                                       (it + 1) * 128],
                                            wo_t[:, ht,
                                                 mc * 512:(mc + 1) * 512],
                                            start=(ht == 0),
                                            stop=(ht == HT - 1),
                                        )
                            for it in range(4):
                                for mc in range(2):
                                    xo = xop.tile([128, 512], f32, tag="xo",
                                                  name="xo")
                                    nc.sync.dma_start(
                                        out=xo,
                                        in_=xo_r[:, ic, it,
                                                 mc * 512:(mc + 1) * 512])
                                    ot = osp.tile([128, 512], f32, tag="ot",
                                                  name="ot")
                                    if obias:
                                        nc.vector.tensor_add(
                                            ot, pos[it * 2 + mc],
                                            bo_t[:, mc * 512:(mc + 1) * 512])
                                        nc.vector.tensor_mul(ot, ot, xo)
                                    else:
                                        nc.vector.tensor_mul(
                                            ot, pos[it * 2 + mc], xo)
                                    nc.sync.dma_start(
                                        out=out.rearrange(
                                            "(ic it p) m -> p ic it m",
                                            p=128, it=4)
                                        [:, ic, it, mc * 512:(mc + 1) * 512],
                                        in_=ot,
                                    )

    nc.compile()
    return nc


# ---------------------------------------------------------------- runner ----

import time as _time

import jax
import jax.numpy as jnp
from jax.sharding import Mesh, NamedSharding, PartitionSpec
from jax.experimental.shard_map import shard_map

from concourse.bass2jax import _bass_exec_p, install_neuronx_cc_hook, partition_id_tensor


class SpmdRunner:
    def __init__(self, nc, n_cores=8):
        install_neuronx_cc_hook()
        self.nc = nc
        self.n_cores = n_cores
        partition_name = nc.partition_id_tensor.name if nc.partition_id_tensor else None
        in_names, out_names, out_avals, zero_outs = [], [], [], []
        for alloc in nc.m.functions[0].allocations:
            if not isinstance(alloc, mybir.MemoryLocationSet):
                continue
            name = alloc.memorylocations[0].name
            if alloc.kind == "ExternalInput":
                if name != partition_name:
                    in_names.append(name)
            elif alloc.kind == "ExternalOutput":
                shape = tuple(alloc.tensor_shape)
                dtype = mybir.dt.np(alloc.dtype)
                out_names.append(name)
                out_avals.append(jax.core.ShapedArray(shape, dtype))
                zero_outs.append(np.zeros(shape, dtype))
        self.in_names, self.out_names = in_names, out_names
        self.out_avals, self.zero_outs = out_avals, zero_outs
        n_params = len(in_names)
        all_names = in_names + out_names
        if partition_name is not None:
            all_names = all_names + [partition_name]

        def _body(*args):
            operands = list(args)
            if partition_name is not None:
                operands.append(partition_id_tensor())
            outs = _bass_exec_p.bind(
                *operands,
                out_avals=tuple(out_avals),
                in_names=tuple(all_names),
                out_names=tuple(out_names),
                lowering_input_output_aliases=(),
                sim_require_finite=True,
                sim_require_nnan=True,
                nc=nc,
            )
            return tuple(outs)

        devices = jax.devices()[:n_cores]
        self.mesh = Mesh(np.asarray(devices), ("core",))
        in_specs = (PartitionSpec("core"),) * (n_params + len(out_names))
        out_specs = (PartitionSpec("core"),) * len(out_names)
        self.sharded = jax.jit(
            shard_map(_body, mesh=self.mesh, in_specs=in_specs,
                      out_specs=out_specs, check_rep=False),
            keep_unused=True,
        )

    def stage_inputs(self, in_maps):
        n = self.n_cores
        concat = [
            np.concatenate([np.asarray(in_maps[c][name]) for c in range(n)], axis=0)
            for name in self.in_names
        ]
        concat += [np.zeros((n * z.shape[0], *z.shape[1:]), z.dtype)
                   for z in self.zero_outs]
        sharding = NamedSharding(self.mesh, PartitionSpec("core"))
        return [jax.device_put(a, sharding) for a in concat]

    def run(self, staged):
        outs = self.sharded(*staged)
        jax.block_until_ready(outs)
        return outs

    def run_numpy(self, staged):
        outs = self.run(staged)
        n = self.n_cores
        return [
            {name: np.asarray(outs[i]).reshape(n, *self.out_avals[i].shape)[c]
             for i, name in enumerate(self.out_names)}
            for c in range(n)
        ]


# ------------------------------------------------------------- host side ----

_CACHE = {}


def _get_runner(reps, vbias, obias):
    key = (reps, vbias, obias)
    if key not in _CACHE:
        nc = _build_nc(reps=reps, vbias=vbias, obias=obias)
        _CACHE[key] = SpmdRunner(nc, NC)
    return _CACHE[key]


def _pad2(v):
    o = np.zeros((2, 128), np.float32)
    o[0] = v[:128]
    o[1, :QK - 128] = v[128:QK]
    return o


def make_in_maps(x, W_hidden, b_hidden, W_qk, b_qk, gamma, beta, W_out, b_out):
    x = np.ascontiguousarray(np.asarray(x, np.float32))
    # attn is stored as fp8e4 scaled by 64 (folded as x8 into the q-side
    # affine; pow-2 scales are exact) and compensated by W_out/64.
    scale = 8.0 / np.sqrt(np.float32(D))
    gq = _pad2(np.asarray(gamma[0], np.float32) * scale)
    bq = _pad2(np.asarray(beta[0], np.float32) * scale)
    gk = _pad2(np.asarray(gamma[1], np.float32))
    bk = _pad2(np.asarray(beta[1], np.float32))
    bqk = _pad2(np.asarray(b_qk, np.float32))
    bg = np.ascontiguousarray(
        np.asarray(b_hidden[H:], np.float32).reshape(HT, 128))
    W_hidden = np.asarray(W_hidden, np.float32).astype(ml_dtypes.bfloat16)
    W_qk = np.asarray(W_qk, np.float32).astype(ml_dtypes.bfloat16)
    W_out = (np.asarray(W_out, np.float32) * (1.0 / 64.0)).astype(
        ml_dtypes.bfloat16)
    W_hidden = np.ascontiguousarray(W_hidden)
    W_qk = np.ascontiguousarray(W_qk)
    W_out = np.ascontiguousarray(W_out)
    bv = np.asarray(b_hidden[:H], np.float32)
    bo = np.asarray(b_out, np.float32)
    vbias = bool(np.any(bv))
    obias = bool(np.any(bo))

    xT16 = np.ascontiguousarray(x.T.astype(ml_dtypes.bfloat16))
    in_maps = []
    for c in range(NC):
        m = {
            "xoT": np.ascontiguousarray(xT16[:, c * R:(c + 1) * R]),
            "x_own": x[c * R:(c + 1) * R],
            "w_h": W_hidden,
            "w_qk": W_qk,
            "w_out": W_out,
            "gq": gq, "bq": bq, "gk": gk, "bk": bk, "bqk": bqk, "bg": bg,
        }
        if vbias:
            m["bv"] = bv
        if obias:
            m["bo"] = bo
        in_maps.append(m)
    return in_maps, vbias, obias


def kernel(x, W_hidden, b_hidden, W_qk, b_qk, gamma, beta, W_out, b_out):
    in_maps, vbias, obias = make_in_maps(
        x, W_hidden, b_hidden, W_qk, b_qk, gamma, beta, W_out, b_out)
    runner = _get_runner(1, vbias, obias)
    staged = runner.stage_inputs(in_maps)
    # Collectives can very rarely deliver corrupted data (transient link
    # glitch observed once in ~10 runs); retry on a non-finite result.
    for attempt in range(3):
        results = runner.run_numpy(staged)
        out = np.concatenate([results[c]["out"] for c in range(NC)], axis=0)
        if np.isfinite(out).all():
            return out
        import sys
        print(f"kernel: non-finite output, retrying ({attempt})",
              file=sys.stderr)
    return out
